# revision 47
# baseline (speedup 1.0000x reference)
"""Trainium2 Bass kernel for nn_CrossAttention_14207751815513.

Single-query cross-attention:
    q = x1 @ Wq.T                 (one query per head)
    k = x2 @ Wk.T ; v = x2 @ Wv.T
    attn_h = softmax(q_h . k_h / sqrt(128))
    out = concat_h(attn_h @ v_h) @ Wo.T + bo

Because there is exactly ONE query, the K and V projections collapse
algebraically (associativity):
    scores_h = x2 @ r_h,  r_h = Wk_h.T q_h / sqrt(128)   -- no k materialization
    out_h    = Wv_h @ (x2.T p_h) / l_h                   -- no v materialization
with p = exp(scores) (logits are small, |s| < ~6, so no max subtraction
is needed) and l_h = sum_s p_h[s].

Sharding: the sequence dim (16384) is split across the 8 NeuronCores
(2048 rows each).  Every quantity that is O(1) in the sequence length
(q, R = [r_1..r_16], the per-head Wv matvec, Wo + bias) lives in the
host-side shard-prep / gather-merge glue; the O(S*C) work runs on
device.  x2 streams in fp8e4 in TWO layouts (x2t: c on partitions for
phase S; x2n: s on partitions for phase T) -- both layouts are
algorithmically required because the PE contracts over the partition
dim, and no on-chip engine can transpose 4M elements fast enough.

v3 changes vs v2 (measured medians are within run noise of each other;
v3 keeps the structurally better schedule):
  * Both input rings are HWDGE (sync + scalar; a gpsimd/SWDGE input
    ring only sustains ~150GB/s vs ~210, measured).  Scalar's queue is
    front-loaded (rsb + x2t first): HWDGE ring-depth backpressure ties
    issue k+3 to completion of issue k, so this frees the Scalar
    ENGINE by ~15us, before the exp needs it (in v2 Scalar was stuck
    issuing its late x2n pieces until ~18.8us).
  * Stream order: ALL of x2t first (balanced across both rings), so
    the S->T hinge (exp + P transposes, ~5.5us serial ACT+DVE) fully
    overlaps the x2n stream.  First x2t piece is 1 chunk (256KB) so S
    matmuls start ~2.5us earlier; each ring's last x2n piece is 1
    chunk so the trailing T matmuls after stream end are short.
  * Keep-warm filler matmuls (junk 32x512 MMs into PSUM rows no real
    instruction touches) between stream pieces so the PE's HAM clock
    gate stays at 2.4GHz (v2 ran most S-phase rounds at 1.2GHz:
    ~420ns vs ~210ns per column-round; throttle_active ~10us).
  * Exp activation table preloaded at t~7.5us via a dummy activation
    (table load is 1.3us; in v2 it sat right before the first exp).
  * One l-copy per engine-half instead of four.

Measured facts that bound further improvement (from NTFF profiles):
  * The 8.25MB/core input stream runs at ~420GB/s aggregate, but SDMA
    engine 15 is ~15% slower than the other 15 (known trn2 quirk), and
    every piece's completion semaphore waits for its slice, so the
    effective stream pace is E15's: ~24us wall on a typical run.
  * A near-empty 8-core Tile program measures 13.5-14.2us: framework
    init + end-of-context drain + semaphore-teardown sweep are a fixed
    ~13.5us of the reported exec time, untouchable from kernel code.
  * 4 accumulation groups striping ONE PSUM bank is numerically fine
    but serializes the 4 concurrent column-group matmul drains on the
    bank write port (~850ns vs ~220ns per round) -- 4 banks kept.
  * The ACT engine cannot move data across partition quadrants
    (in[32:48] -> out[16:32] fails BIR verification), so the P->PT
    transpose cannot be halved by compacting exp outputs.
  * Run-to-run variance is +-3-4us (P-state derates every engine ~15%
    on some runs; E15 lag varies), dominating remaining micro-wins.

Sync-wait note: this backend disables DynamicDMA, so every HW-DGE DMA
lowers to a pseudo-direct DMA that supports at most ONE semaphore wait
("Too many sync wait commands" in walrus codegen otherwise).  Input
stream DMAs use fresh buffers (no WAR/WAW waits), so their only wait
is an occasional sem-slot-recycle wait.  The output DMA (which carries
a RAW wait on the Scalar copies) runs on gpsimd/SWDGE, which tolerates
multiple waits.  The end-of-context Drain gets a sem wait for every
proc the SP engine hasn't directly observed, so sync nops observe each
DMA and each engine's last instruction.
"""

import sys

for _p in ("/root/.axon_site/_ro/trn_rl_repo", "/opt/trn_rl_repo"):
    if _p not in sys.path:
        sys.path.append(_p)

import numpy as np
import ml_dtypes

import concourse.bass as bass
import concourse.tile as tile
from concourse import mybir
from concourse.bass_utils import run_bass_kernel_spmd
from concourse.tile_rust import add_dep_helper

NCORES = 8
S_FULL = 16384
C = 2048           # input feature dim (both x1 and x2)
H = 16             # heads
J = 128            # head dim (K_DIM == V_DIM == 128)
HJ = H * J         # 2048
ODIM = 512
S_LOC = S_FULL // NCORES   # 2048 sequence rows per core

BF = mybir.dt.bfloat16
F32 = mybir.dt.float32
F8 = mybir.dt.float8e4
INV_SQRT_K = 1.0 / float(np.sqrt(128.0))
RSCALE = 64.0      # host multiplies R by this; Exp activation divides

NB = 512                    # PSUM bank free-dim (f32 columns)
CH = C // 128               # 16 chunks of 128 along any 2048 dim
NG = 4                      # column-tile groups
RW = 32                     # R columns per chunk (16 heads + 16 zero pad)

_F8_NP = ml_dtypes.float8_e4m3fn

PSCALE = 1.0

# 4 accumulation groups striping one PSUM bank is numerically fine on HW,
# but the 4 concurrent column-group matmuls then contend for the single
# bank write port: ~850ns per 4-MM round vs ~220ns with 4 banks (measured).
# Keep one bank per group.
SINGLE_BANK = False

# Stream schedule: (ring, first_chunk, n_chunks) in expected ARRIVAL order.
# ring 0 = sync, ring 1 = scalar -- both HWDGE (the gpsimd/SWDGE ring only
# sustains ~150GB/s vs ~210 for each HWDGE ring; measured).  Bytes balanced
# ~4.06MB per ring.  Scalar's queue is front-loaded (rsb + x2t first) so ring
# depth backpressure (issue k+3 waits completion of issue k) frees the Scalar
# ENGINE by ~15us, before the exp needs it.  ALL of x2t precedes x2n so the
# S->T hinge overlaps the x2n stream; first x2t piece is 1 chunk for an early
# PE start; the trailing pieces are small so post-stream matmul work is short.
# (An experiment packing x2t in 120-row chunks to halve the slow SDMA engine
# 15's bytes regressed +12us: sub-128-partition DMAs fall off the efficient
# descriptor path.  128-row chunks throughout.)
# Last x2t piece is 1 chunk: the S tail after the final x2t completion is
# then one matmul round, so the serial exp+transpose hinge starts ~0.6us
# earlier (it ties with the x2n stream end on the critical path).
XT_SCHED = [(0, 0, 1), (1, 1, 4), (0, 5, 4), (1, 9, 3), (0, 12, 3), (1, 15, 1)]
XN_SCHED = [(0, 0, 4), (1, 4, 4), (0, 8, 3), (1, 11, 3), (0, 14, 1), (1, 15, 1)]

import os as _os

N_FILL = int(_os.environ.get("KFILL", "2"))   # keep-warm fillers per gap


def _build_program() -> bass.Bass:
    nc = bass.Bass()
    # x2t/x2n are packed partition-major on the host ([p, chunk, col]) so a
    # multi-chunk stream DMA folds to ONE contiguous descriptor per partition.
    t_in = {
        "rsb": nc.dram_tensor("rsb", [J, CH, RW], BF, kind="ExternalInput"),
        "x2t": nc.dram_tensor("x2t", [J, CH, S_LOC], F8, kind="ExternalInput"),
        "x2n": nc.dram_tensor("x2n", [J, CH, C], F8, kind="ExternalInput"),
    }
    t_out = {
        "tt": nc.dram_tensor("tt", [J, NB + 1], BF, kind="ExternalOutput"),
    }

    rsb_d = t_in["rsb"][:, :, :]
    x2t_v = t_in["x2t"][:, :, :]
    x2n_v = t_in["x2n"][:, :, :]
    tt_out = t_out["tt"][:, :]

    with tile.TileContext(nc) as tc:
        with (
            tc.tile_pool(name="singles", bufs=1) as singles,
            tc.tile_pool(name="sa", bufs=1) as sa,
            tc.tile_pool(name="sb", bufs=1) as sbp,
            tc.tile_pool(name="psbig", bufs=8, space="PSUM") as psbig,
        ):
            stream_dmas = []    # input-stream DMAs: funneled EARLY (overlapped)

            # ---- issue every stream DMA up front (both rings) --------------
            Rsb = singles.tile([J, CH, RW], BF)
            stream_dmas.append(nc.scalar.dma_start(out=Rsb, in_=rsb_d))

            s_tiles = []        # (first_cc, n_cc, K, tile)
            for ci, (ring, a, k) in enumerate(XT_SCHED):
                eng = nc.sync if ring == 0 else nc.scalar
                xt = sa.tile([128, k, S_LOC], F8, tag=f"sa_{ci}")
                stream_dmas.append(eng.dma_start(out=xt, in_=x2t_v[:, a : a + k, :]))
                s_tiles.append((a, k, J, xt))
            t_tiles = []
            for ci, (ring, a, k) in enumerate(XN_SCHED):
                eng = nc.sync if ring == 0 else nc.scalar
                xn = sbp.tile([128, k, C], F8, tag=f"sb_{ci}")
                stream_dmas.append(eng.dma_start(out=xn, in_=x2n_v[:, a : a + k, :]))
                t_tiles.append((a, k, xn))

            # Early drain-funnel for stream DMAs: sync nops that fire as each
            # piece lands (hidden under the stream); the end-of-context Drain
            # itself cannot carry a wait per DMA.
            for t in stream_dmas:
                n = nc.sync.nop(nofuse=True, hint="dep")
                add_dep_helper(n.ins, t.ins, reason="drain-funnel-early")

            # Exp table preload: a dummy activation as soon as rsb lands, so
            # the 1.3us ACT_TABLE_LOAD overlaps the stream instead of sitting
            # in front of the real exp.
            dummy = singles.tile([1, 1], F32)
            nc.scalar.activation(
                out=dummy,
                in_=Rsb[0:1, 0, 0:1],
                func=mybir.ActivationFunctionType.Exp,
                scale=1.0 / RSCALE,
            )

            # PSUM banks: scores + t (striped single-bank or 4 banks) + junk
            if SINGLE_BANK:
                ps_s_t = psbig.tile([J, NB], F32, tag="big", name="ps_s")
                ps_t_t = psbig.tile([J, NB], F32, tag="big", name="ps_t")
                ps_s = [ps_s_t] * NG
                ps_t = [ps_t_t] * NG
                junk = psbig.tile([J, NB], F32, tag="big", name="ps_junk")
            else:
                ps_s = [
                    psbig.tile([J, NB], F32, tag="big", name=f"ps_s{g}")
                    for g in range(NG)
                ]
                ps_t = [
                    psbig.tile([J, NB], F32, tag="big", name=f"ps_t{g}")
                    for g in range(NG)
                ]
                junk = None

            def filler(rhs_ap, n=1, dep=None):
                """Keep-warm junk matmul(s) into partitions 32-63 of the
                ps_t[0] bank -- rows no real instruction ever touches, so a
                filler carries NO data dependencies (its rhs was always
                waited on by a preceding real MM).  Runs in PE FIFO order;
                keeps the HAM clock gate at 2.4GHz across stream-wait gaps."""
                if junk is not None:
                    jt, row0 = junk, 0
                else:
                    jt, row0 = ps_t[0], 32
                w = rhs_ap.free_size()
                kk = rhs_ap.partition_size()
                for _ in range(n):
                    i = nc.tensor.matmul(
                        jt[row0 : row0 + 32, :w],
                        lhsT=Rsb[0:kk, 0, :],
                        rhs=rhs_ap,
                        start=True,
                        stop=True,
                        tile_position=(0, row0),
                        skip_group_check=True,
                    )
                    if dep is not None:
                        add_dep_helper(i.ins, dep.ins, reason="keepwarm-spread")
                return i

            # PE warmup while the first x2t piece is in flight (dep: rsb DMA
            # via reading Rsb).  ~4 x 0.43us of junk work starting ~7.5us.
            if N_FILL:
                filler(Rsb[:, 4:8, :], n=2)

            # ---- phase S: scores[h, 512g+j] = sum_c R[c, h] x2t[c, 512g+j] -
            # col-group g owns s-block g; its accumulator is the 16-row
            # stripe [32g:32g+16] of its own PSUM bank.
            NCH_S = CH       # 16 score-contraction chunks of 128
            n_done = 0
            for a, k, kpart, xt in s_tiles:
                for gg in range(k):
                    cc = a + gg
                    n_done += 1
                    for g in range(NG):
                        nc.tensor.matmul(
                            ps_s[g][32 * g : 32 * g + 16, :],
                            lhsT=Rsb[0:kpart, cc, :H],
                            rhs=xt[:, gg, g * NB : (g + 1) * NB],
                            start=(n_done == 1),
                            stop=(n_done == NCH_S),
                            tile_position=(0, 32 * g),
                        )
                # keep-warm between stream pieces
                if n_done < NCH_S and N_FILL:
                    filler(xt[:, 0, 0:512], n=N_FILL)

            # ---- exp (scale un-does RSCALE; |logit| < ~6) ------------------
            # Four ACTIVATEs (one per score bank), in/out at the SAME
            # partition base (the ACT engine rejects cross-quadrant partition
            # moves).  accum_out gives the per-head row sums l for free.
            Psb = singles.tile([J, 4, 128], BF)   # [32g+h, s_hi, s_lo]
            lacc = singles.tile([J, 1], F32)
            for g in range(NG):
                nc.scalar.activation(
                    out=Psb[32 * g : 32 * g + 16, :, :],
                    in_=ps_s[g][32 * g : 32 * g + 16, :],
                    func=mybir.ActivationFunctionType.Exp,
                    scale=1.0 / RSCALE,
                    accum_out=lacc[32 * g : 32 * g + 16, :],
                )

            tt_sb = singles.tile([J, NB + 1], BF)
            # (scalar l-copy here; the VECTOR l-copy is emitted after the
            # transposes so it can't interleave into -- and delay -- the
            # transpose chain that gates the last T matmuls.)
            nc.scalar.copy(out=tt_sb[0:48, NB : NB + 1], in_=lacc[0:48, :])

            # ---- transpose P -> PT [128, 16 schunk, 32(h pad)] bf16 --------
            # DVE blockwise 32x32 transposes straight from the exp output
            # (SBUF->SBUF, off the PE queue).  One instruction per (group g,
            # partition quarter m) covers 4 blocks via the strided [4, 32]
            # free pattern: chunk sc = 4g + j4, s = 128*sc + 32m + i.  Rows
            # 32g+16:32g+32 of Psb are unwritten junk; they land in PT
            # columns 16:32, which the T matmuls never read.
            PT = singles.tile([J, CH, 32], BF)
            i_trs = []
            for g in range(NG):
                for m in range(4):
                    i_trs.append(
                        nc.vector.transpose(
                            out=PT[32 * m : 32 * m + 32, 4 * g : 4 * g + 4, :],
                            in_=Psb[32 * g : 32 * g + 32, :, 32 * m : 32 * m + 32],
                        )
                    )

            # vector half of the l column, AFTER the transposes in DVE FIFO
            # (hidden under the T-phase stream; same engine as the final
            # casts so output DMA 2 carries exactly one sem wait).
            nc.vector.tensor_copy(out=tt_sb[64:112, NB : NB + 1], in_=lacc[64:112, :])

            # (No fillers across the hinge: the exp->transpose->first-T-MM
            # gap is ~2us, under the HAM 3.4us re-throttle window, and a
            # filler dep'd on a transpose would both block the T matmuls in
            # PE FIFO order and push the matmul past the sync-wait limit.)

            # ---- phase T: t[h, 512g+j] = sum_s PT[s, h] x2n[s, 512g+j] -----
            i_pe = None
            n_done = 0
            for a, k, xn in t_tiles:
                for gg in range(k):
                    sc = a + gg
                    n_done += 1
                    for g in range(NG):
                        i_pe = nc.tensor.matmul(
                            ps_t[g][32 * g : 32 * g + 16, :],
                            lhsT=PT[:, sc, :H],
                            rhs=xn[:, gg, g * NB : (g + 1) * NB],
                            start=(n_done == 1),
                            stop=(n_done == CH),
                            tile_position=(0, 32 * g),
                        )
                # keep-warm between stream pieces (reads this piece's data,
                # so it never blocks later PE work on a not-yet-needed dep)
                if n_done < CH and N_FILL:
                    filler(xn[:, 0, 0:512], n=N_FILL)

            # output: cast copies (PSUM f32 -> SBUF bf16), scalar for the low
            # partition half / vector for the high half, then one SWDGE DMA
            # per half so each carries exactly one RAW wait (pseudo-direct
            # DMAs support at most one sem wait).
            cps = []
            for g in range(NG):
                cp = nc.scalar.copy if g < 2 else nc.vector.tensor_copy
                cps.append(
                    cp(
                        out=tt_sb[32 * g : 32 * g + 16, :NB],
                        in_=ps_t[g][32 * g : 32 * g + 16, :],
                    )
                )
            last_cps = [cps[1], cps[3]]   # scalar-last, vector-last
            i_out = nc.gpsimd.dma_start(out=tt_out[0:64, :], in_=tt_sb[0:64, :])
            i_out2 = nc.gpsimd.dma_start(out=tt_out[64:128, :], in_=tt_sb[64:128, :])

            # ---- drain-funnel epilogue (see sync-wait note above): one nop
            # per un-observed proc -- each engine's LAST instruction plus the
            # output DMAs (stream DMAs were funneled early).
            for t in last_cps + [i_pe, i_out, i_out2]:
                n = nc.sync.nop(nofuse=True, hint="dep")
                add_dep_helper(n.ins, t.ins, reason="drain-funnel")

    return nc


_NC_CACHE = None


def _get_nc() -> bass.Bass:
    global _NC_CACHE
    if _NC_CACHE is None:
        _NC_CACHE = _build_program()
    return _NC_CACHE


def _prep_in_maps(x1, x2, Wq, Wk):
    x1 = np.asarray(x1, np.float32)
    x2 = np.asarray(x2, np.float32)
    Wq = np.asarray(Wq, np.float32)
    Wk = np.asarray(Wk, np.float32)

    # R[c, h] = sum_j Wk[h*128+j, c] q[h*128+j] / sqrt(128),  scaled by RSCALE
    q = (Wq @ x1) * (INV_SQRT_K * RSCALE)                       # [2048]
    R = np.einsum("hj,hjc->ch", q.reshape(H, J), Wk.reshape(H, J, C))
    rsb = np.zeros((J, CH, RW), np.float32)
    rsb[:, :, :H] = R.reshape(CH, 128, H).transpose(1, 0, 2)
    rsb = rsb.astype(ml_dtypes.bfloat16)                        # [128, 16, 32]

    in_maps = []
    for c in range(NCORES):
        shard = x2[c * S_LOC : (c + 1) * S_LOC]                 # [2048, 2048]
        # packed partition-major: x2t[p, cc, s] = shard.T[cc*128+p, s],
        #                         x2n[p, sc, c] = shard[sc*128+p, c]
        x2t_c = np.ascontiguousarray(
            shard.T.reshape(CH, 128, S_LOC).transpose(1, 0, 2)
        ).astype(_F8_NP)
        x2n_c = np.ascontiguousarray(
            shard.reshape(CH, 128, C).transpose(1, 0, 2)
        ).astype(_F8_NP)
        in_maps.append({"rsb": rsb, "x2t": x2t_c, "x2n": x2n_c})
    return in_maps


def _merge(results, Wv, Wo, bo):
    Wv = np.asarray(Wv, np.float32)
    Wo = np.asarray(Wo, np.float32)
    bo = np.asarray(bo, np.float32)
    t_tot = np.zeros((H, C), np.float64)
    l_tot = np.zeros(H, np.float64)
    L_ROW = [0, 32, 64, 96]   # where group g's l column lives (see l-copies)
    for r in results:
        tt = r["tt"].astype(np.float64)                         # [128, 513]
        for g in range(NG):
            t_tot[:, g * NB : (g + 1) * NB] += tt[32 * g : 32 * g + H, :NB]
            l_tot += tt[L_ROW[g] : L_ROW[g] + H, NB] * PSCALE
    tn = t_tot / l_tot[:, None]                                 # [16, 2048]
    u = np.einsum("hc,hjc->hj", tn, Wv.astype(np.float64).reshape(H, J, C))
    out = u.reshape(HJ) @ Wo.T.astype(np.float64) + bo.astype(np.float64)
    return out.astype(np.float32).reshape(1, ODIM)


def kernel(x1, x2, Wq, Wk, Wv, Wo, bo):
    nc = _get_nc()
    in_maps = _prep_in_maps(x1, x2, Wq, Wk)
    res = run_bass_kernel_spmd(nc, in_maps, list(range(NCORES)))
    return _merge(res.results, Wv, Wo, bo)


def run_traced(x1, x2, Wq, Wk, Wv, Wo, bo, **trace_kwargs):
    """Like kernel() but returns (output, BassKernelResults) with NTFF trace."""
    nc = _get_nc()
    in_maps = _prep_in_maps(x1, x2, Wq, Wk)
    res = run_bass_kernel_spmd(
        nc, in_maps, list(range(NCORES)), trace=True, **trace_kwargs
    )
    return _merge(res.results, Wv, Wo, bo), res


# revision 48
# speedup vs baseline: 1.1549x; 1.1549x over previous
"""Trainium2 Bass kernel for nn_CrossAttention_14207751815513.

Single-query cross-attention:
    q = x1 @ Wq.T                 (one query per head)
    k = x2 @ Wk.T ; v = x2 @ Wv.T
    attn_h = softmax(q_h . k_h / sqrt(128))
    out = concat_h(attn_h @ v_h) @ Wo.T + bo

Because there is exactly ONE query, the K and V projections collapse
algebraically (associativity):
    scores_h = x2 @ r_h,  r_h = Wk_h.T q_h / sqrt(128)   -- no k materialization
    out_h    = Wv_h @ (x2.T p_h) / l_h                   -- no v materialization
with p = exp(scores) (logits are small, |s| < ~6, so no max subtraction
is needed) and l_h = sum_s p_h[s].

Sharding: the sequence dim (16384) is split across the 8 NeuronCores
(2048 rows each).  Every quantity that is O(1) in the sequence length
(q, R = [r_1..r_16], the per-head Wv matvec, Wo + bias) lives in the
host-side shard-prep / gather-merge glue; the O(S*C) work runs on
device.  x2 streams in fp8e4 in TWO layouts (x2t: c on partitions for
phase S; x2n: s on partitions for phase T) -- both layouts are
algorithmically required because the PE contracts over the partition
dim, and no on-chip engine can transpose 4M elements fast enough.

v3 changes vs v2 (measured medians are within run noise of each other;
v3 keeps the structurally better schedule):
  * Both input rings are HWDGE (sync + scalar; a gpsimd/SWDGE input
    ring only sustains ~150GB/s vs ~210, measured).  Scalar's queue is
    front-loaded (rsb + x2t first): HWDGE ring-depth backpressure ties
    issue k+3 to completion of issue k, so this frees the Scalar
    ENGINE by ~15us, before the exp needs it (in v2 Scalar was stuck
    issuing its late x2n pieces until ~18.8us).
  * Stream order: ALL of x2t first (balanced across both rings), so
    the S->T hinge (exp + P transposes, ~5.5us serial ACT+DVE) fully
    overlaps the x2n stream.  First x2t piece is 1 chunk (256KB) so S
    matmuls start ~2.5us earlier; each ring's last x2n piece is 1
    chunk so the trailing T matmuls after stream end are short.
  * Keep-warm filler matmuls (junk 32x512 MMs into PSUM rows no real
    instruction touches) between stream pieces so the PE's HAM clock
    gate stays at 2.4GHz (v2 ran most S-phase rounds at 1.2GHz:
    ~420ns vs ~210ns per column-round; throttle_active ~10us).
  * Exp activation table preloaded at t~7.5us via a dummy activation
    (table load is 1.3us; in v2 it sat right before the first exp).
  * One l-copy per engine-half instead of four.

Measured facts that bound further improvement (from NTFF profiles):
  * The 8.25MB/core input stream runs at ~420GB/s aggregate, but SDMA
    engine 15 is ~15% slower than the other 15 (known trn2 quirk), and
    every piece's completion semaphore waits for its slice, so the
    effective stream pace is E15's: ~24us wall on a typical run.
  * A near-empty 8-core Tile program measures 13.5-14.2us: framework
    init + end-of-context drain + semaphore-teardown sweep are a fixed
    ~13.5us of the reported exec time, untouchable from kernel code.
  * 4 accumulation groups striping ONE PSUM bank is numerically fine
    but serializes the 4 concurrent column-group matmul drains on the
    bank write port (~850ns vs ~220ns per round) -- 4 banks kept.
  * The ACT engine cannot move data across partition quadrants
    (in[32:48] -> out[16:32] fails BIR verification), so the P->PT
    transpose cannot be halved by compacting exp outputs.
  * Run-to-run variance is +-3-4us (P-state derates every engine ~15%
    on some runs; E15 lag varies), dominating remaining micro-wins.

Sync-wait note: this backend disables DynamicDMA, so every HW-DGE DMA
lowers to a pseudo-direct DMA that supports at most ONE semaphore wait
("Too many sync wait commands" in walrus codegen otherwise).  Input
stream DMAs use fresh buffers (no WAR/WAW waits), so their only wait
is an occasional sem-slot-recycle wait.  The output DMA (which carries
a RAW wait on the Scalar copies) runs on gpsimd/SWDGE, which tolerates
multiple waits.  The end-of-context Drain gets a sem wait for every
proc the SP engine hasn't directly observed, so sync nops observe each
DMA and each engine's last instruction.
"""

import sys

for _p in ("/root/.axon_site/_ro/trn_rl_repo", "/opt/trn_rl_repo"):
    if _p not in sys.path:
        sys.path.append(_p)

import numpy as np
import ml_dtypes

import concourse.bass as bass
import concourse.tile as tile
from concourse import mybir
from concourse.bass_utils import run_bass_kernel_spmd
from concourse.tile_rust import add_dep_helper

NCORES = 8
S_FULL = 16384
C = 2048           # input feature dim (both x1 and x2)
H = 16             # heads
J = 128            # head dim (K_DIM == V_DIM == 128)
HJ = H * J         # 2048
ODIM = 512
S_LOC = S_FULL // NCORES   # 2048 sequence rows per core

BF = mybir.dt.bfloat16
F32 = mybir.dt.float32
F8 = mybir.dt.float8e4
INV_SQRT_K = 1.0 / float(np.sqrt(128.0))
RSCALE = 64.0      # host multiplies R by this; Exp activation divides

NB = 512                    # PSUM bank free-dim (f32 columns)
CH = C // 128               # 16 chunks of 128 along any 2048 dim
NG = 4                      # column-tile groups
RW = 32                     # R columns per chunk (16 heads + 16 zero pad)

_F8_NP = ml_dtypes.float8_e4m3fn

PSCALE = 1.0

# 4 accumulation groups striping one PSUM bank is numerically fine on HW,
# but the 4 concurrent column-group matmuls then contend for the single
# bank write port: ~850ns per 4-MM round vs ~220ns with 4 banks (measured).
# Keep one bank per group.
SINGLE_BANK = False

# Stream schedule: (ring, first_chunk, n_chunks) in expected ARRIVAL order.
# ring 0 = sync, ring 1 = scalar -- both HWDGE (the gpsimd/SWDGE ring only
# sustains ~150GB/s vs ~210 for each HWDGE ring; measured).  Bytes balanced
# ~4.06MB per ring.  Scalar's queue is front-loaded (rsb + x2t first) so ring
# depth backpressure (issue k+3 waits completion of issue k) frees the Scalar
# ENGINE by ~15us, before the exp needs it.  ALL of x2t precedes x2n so the
# S->T hinge overlaps the x2n stream; first x2t piece is 1 chunk for an early
# PE start; the trailing pieces are small so post-stream matmul work is short.
# (An experiment packing x2t in 120-row chunks to halve the slow SDMA engine
# 15's bytes regressed +12us: sub-128-partition DMAs fall off the efficient
# descriptor path.  128-row chunks throughout.)
# Last x2t piece is 1 chunk: the S tail after the final x2t completion is
# then one matmul round, so the serial exp+transpose hinge starts ~0.6us
# earlier (it ties with the x2n stream end on the critical path).
XT_SCHED = [(0, 0, 1), (1, 1, 4), (0, 5, 4), (0, 9, 3), (1, 12, 4)]
XN_SCHED = [(0, 0, 4), (1, 4, 4), (0, 8, 3), (1, 11, 3), (0, 14, 1), (1, 15, 1)]

import os as _os

N_FILL = int(_os.environ.get("KFILL", "2"))   # keep-warm fillers per gap


def _build_program() -> bass.Bass:
    nc = bass.Bass()
    # x2t/x2n are packed partition-major on the host ([p, chunk, col]) so a
    # multi-chunk stream DMA folds to ONE contiguous descriptor per partition.
    t_in = {
        "rsb": nc.dram_tensor("rsb", [J, CH, RW], BF, kind="ExternalInput"),
        "x2t": nc.dram_tensor("x2t", [J, CH, S_LOC], F8, kind="ExternalInput"),
        "x2n": nc.dram_tensor("x2n", [J, CH, C], F8, kind="ExternalInput"),
    }
    t_out = {
        "tt": nc.dram_tensor("tt", [J, NB + 1], BF, kind="ExternalOutput"),
    }

    rsb_d = t_in["rsb"][:, :, :]
    x2t_v = t_in["x2t"][:, :, :]
    x2n_v = t_in["x2n"][:, :, :]
    tt_out = t_out["tt"][:, :]

    with tile.TileContext(nc) as tc:
        with (
            tc.tile_pool(name="singles", bufs=1) as singles,
            tc.tile_pool(name="sa", bufs=1) as sa,
            tc.tile_pool(name="sb", bufs=1) as sbp,
            tc.tile_pool(name="psbig", bufs=8, space="PSUM") as psbig,
        ):
            stream_dmas = []    # input-stream DMAs: funneled EARLY (overlapped)

            # ---- issue every stream DMA up front (both rings) --------------
            Rsb = singles.tile([J, CH, RW], BF)
            stream_dmas.append(nc.scalar.dma_start(out=Rsb, in_=rsb_d))

            s_tiles = []        # (first_cc, n_cc, K, tile)
            for ci, (ring, a, k) in enumerate(XT_SCHED):
                eng = nc.sync if ring == 0 else nc.scalar
                xt = sa.tile([128, k, S_LOC], F8, tag=f"sa_{ci}")
                stream_dmas.append(eng.dma_start(out=xt, in_=x2t_v[:, a : a + k, :]))
                s_tiles.append((a, k, J, xt))
            t_tiles = []
            for ci, (ring, a, k) in enumerate(XN_SCHED):
                eng = nc.sync if ring == 0 else nc.scalar
                xn = sbp.tile([128, k, C], F8, tag=f"sb_{ci}")
                stream_dmas.append(eng.dma_start(out=xn, in_=x2n_v[:, a : a + k, :]))
                t_tiles.append((a, k, xn))

            # Early drain-funnel for stream DMAs: sync nops that fire as each
            # piece lands (hidden under the stream); the end-of-context Drain
            # itself cannot carry a wait per DMA.
            for t in stream_dmas:
                n = nc.sync.nop(nofuse=True, hint="dep")
                add_dep_helper(n.ins, t.ins, reason="drain-funnel-early")

            # Exp table preload: a dummy activation as soon as rsb lands, so
            # the 1.3us ACT_TABLE_LOAD overlaps the stream instead of sitting
            # in front of the real exp.
            dummy = singles.tile([1, 1], F32)
            nc.scalar.activation(
                out=dummy,
                in_=Rsb[0:1, 0, 0:1],
                func=mybir.ActivationFunctionType.Exp,
                scale=1.0 / RSCALE,
            )

            # PSUM banks: scores + t (striped single-bank or 4 banks) + junk
            if SINGLE_BANK:
                ps_s_t = psbig.tile([J, NB], F32, tag="big", name="ps_s")
                ps_t_t = psbig.tile([J, NB], F32, tag="big", name="ps_t")
                ps_s = [ps_s_t] * NG
                ps_t = [ps_t_t] * NG
                junk = psbig.tile([J, NB], F32, tag="big", name="ps_junk")
            else:
                ps_s = [
                    psbig.tile([J, NB], F32, tag="big", name=f"ps_s{g}")
                    for g in range(NG)
                ]
                ps_t = [
                    psbig.tile([J, NB], F32, tag="big", name=f"ps_t{g}")
                    for g in range(NG)
                ]
                junk = None

            def filler(rhs_ap, n=1, dep=None):
                """Keep-warm junk matmul(s) into partitions 32-63 of the
                ps_t[0] bank -- rows no real instruction ever touches, so a
                filler carries NO data dependencies (its rhs was always
                waited on by a preceding real MM).  Runs in PE FIFO order;
                keeps the HAM clock gate at 2.4GHz across stream-wait gaps."""
                if junk is not None:
                    jt, row0 = junk, 0
                else:
                    jt, row0 = ps_t[0], 32
                w = rhs_ap.free_size()
                kk = rhs_ap.partition_size()
                for _ in range(n):
                    i = nc.tensor.matmul(
                        jt[row0 : row0 + 32, :w],
                        lhsT=Rsb[0:kk, 0, :],
                        rhs=rhs_ap,
                        start=True,
                        stop=True,
                        tile_position=(0, row0),
                        skip_group_check=True,
                    )
                    if dep is not None:
                        add_dep_helper(i.ins, dep.ins, reason="keepwarm-spread")
                return i

            # PE warmup while the first x2t piece is in flight (dep: rsb DMA
            # via reading Rsb).  ~4 x 0.43us of junk work starting ~7.5us.
            if N_FILL:
                filler(Rsb[:, 4:8, :], n=4)

            # ---- phase S: scores[h, 512g+j] = sum_c R[c, h] x2t[c, 512g+j] -
            # col-group g owns s-block g; its accumulator is the 16-row
            # stripe [32g:32g+16] of its own PSUM bank.
            NCH_S = CH       # 16 score-contraction chunks of 128
            n_done = 0
            for a, k, kpart, xt in s_tiles:
                for gg in range(k):
                    cc = a + gg
                    n_done += 1
                    for g in range(NG):
                        nc.tensor.matmul(
                            ps_s[g][32 * g : 32 * g + 16, :],
                            lhsT=Rsb[0:kpart, cc, :H],
                            rhs=xt[:, gg, g * NB : (g + 1) * NB],
                            start=(n_done == 1),
                            stop=(n_done == NCH_S),
                            tile_position=(0, 32 * g),
                        )
                # keep-warm between stream pieces
                if n_done < NCH_S and N_FILL:
                    filler(xt[:, 0, 0:512], n=N_FILL)

            # ---- exp (scale un-does RSCALE; |logit| < ~6) ------------------
            # Four ACTIVATEs (one per score bank), in/out at the SAME
            # partition base (the ACT engine rejects cross-quadrant partition
            # moves).  accum_out gives the per-head row sums l for free.
            Psb = singles.tile([J, 4, 128], BF)   # [32g+h, s_hi, s_lo]
            lacc = singles.tile([J, 1], F32)
            for g in range(NG):
                nc.scalar.activation(
                    out=Psb[32 * g : 32 * g + 16, :, :],
                    in_=ps_s[g][32 * g : 32 * g + 16, :],
                    func=mybir.ActivationFunctionType.Exp,
                    scale=1.0 / RSCALE,
                    accum_out=lacc[32 * g : 32 * g + 16, :],
                )

            tt_sb = singles.tile([J, NB + 1], BF)
            # (scalar l-copy here; the VECTOR l-copy is emitted after the
            # transposes so it can't interleave into -- and delay -- the
            # transpose chain that gates the last T matmuls.)
            nc.scalar.copy(out=tt_sb[0:48, NB : NB + 1], in_=lacc[0:48, :])
            nc.vector.tensor_copy(out=tt_sb[64:112, NB : NB + 1], in_=lacc[64:112, :])

            # ---- transpose P -> PT [128, 16 schunk, 32(h pad)] bf16 --------
            # DVE blockwise 32x32 transposes straight from the exp output
            # (SBUF->SBUF, off the PE queue).  One instruction per (group g,
            # partition quarter m) covers 4 blocks via the strided [4, 32]
            # free pattern: chunk sc = 4g + j4, s = 128*sc + 32m + i.  Rows
            # 32g+16:32g+32 of Psb are unwritten junk; they land in PT
            # columns 16:32, which the T matmuls never read.
            PT = singles.tile([J, CH, 32], BF)
            i_trs = []
            for g in range(NG):
                for m in range(4):
                    i_trs.append(
                        nc.vector.transpose(
                            out=PT[32 * m : 32 * m + 32, 4 * g : 4 * g + 4, :],
                            in_=Psb[32 * g : 32 * g + 32, :, 32 * m : 32 * m + 32],
                        )
                    )

            # (No fillers across the hinge: the exp->transpose->first-T-MM
            # gap is ~2us, under the HAM 3.4us re-throttle window, and a
            # filler dep'd on a transpose would both block the T matmuls in
            # PE FIFO order and push the matmul past the sync-wait limit.)

            # ---- phase T: t[h, 512g+j] = sum_s PT[s, h] x2n[s, 512g+j] -----
            i_pe = None
            n_done = 0
            for a, k, xn in t_tiles:
                for gg in range(k):
                    sc = a + gg
                    n_done += 1
                    for g in range(NG):
                        i_pe = nc.tensor.matmul(
                            ps_t[g][32 * g : 32 * g + 16, :],
                            lhsT=PT[:, sc, :H],
                            rhs=xn[:, gg, g * NB : (g + 1) * NB],
                            start=(n_done == 1),
                            stop=(n_done == CH),
                            tile_position=(0, 32 * g),
                        )
                # keep-warm between stream pieces (reads this piece's data,
                # so it never blocks later PE work on a not-yet-needed dep)
                if n_done < CH and N_FILL:
                    filler(xn[:, 0, 0:512], n=N_FILL)

            # output: cast copies (PSUM f32 -> SBUF bf16), scalar for the low
            # partition half / vector for the high half, then one SWDGE DMA
            # per half so each carries exactly one RAW wait (pseudo-direct
            # DMAs support at most one sem wait).
            cps = []
            for g in range(NG):
                cp = nc.scalar.copy if g < 2 else nc.vector.tensor_copy
                cps.append(
                    cp(
                        out=tt_sb[32 * g : 32 * g + 16, :NB],
                        in_=ps_t[g][32 * g : 32 * g + 16, :],
                    )
                )
            last_cps = [cps[1], cps[3]]   # scalar-last, vector-last
            i_out = nc.gpsimd.dma_start(out=tt_out[0:64, :], in_=tt_sb[0:64, :])
            i_out2 = nc.gpsimd.dma_start(out=tt_out[64:128, :], in_=tt_sb[64:128, :])

            # ---- drain-funnel epilogue (see sync-wait note above): one nop
            # per un-observed proc -- each engine's LAST instruction plus the
            # output DMAs (stream DMAs were funneled early).
            for t in last_cps + [i_pe, i_out, i_out2]:
                n = nc.sync.nop(nofuse=True, hint="dep")
                add_dep_helper(n.ins, t.ins, reason="drain-funnel")

    return nc


_NC_CACHE = None


def _get_nc() -> bass.Bass:
    global _NC_CACHE
    if _NC_CACHE is None:
        _NC_CACHE = _build_program()
    return _NC_CACHE


def _prep_in_maps(x1, x2, Wq, Wk):
    x1 = np.asarray(x1, np.float32)
    x2 = np.asarray(x2, np.float32)
    Wq = np.asarray(Wq, np.float32)
    Wk = np.asarray(Wk, np.float32)

    # R[c, h] = sum_j Wk[h*128+j, c] q[h*128+j] / sqrt(128),  scaled by RSCALE
    q = (Wq @ x1) * (INV_SQRT_K * RSCALE)                       # [2048]
    R = np.einsum("hj,hjc->ch", q.reshape(H, J), Wk.reshape(H, J, C))
    rsb = np.zeros((J, CH, RW), np.float32)
    rsb[:, :, :H] = R.reshape(CH, 128, H).transpose(1, 0, 2)
    rsb = rsb.astype(ml_dtypes.bfloat16)                        # [128, 16, 32]

    in_maps = []
    for c in range(NCORES):
        shard = x2[c * S_LOC : (c + 1) * S_LOC]                 # [2048, 2048]
        # packed partition-major: x2t[p, cc, s] = shard.T[cc*128+p, s],
        #                         x2n[p, sc, c] = shard[sc*128+p, c]
        x2t_c = np.ascontiguousarray(
            shard.T.reshape(CH, 128, S_LOC).transpose(1, 0, 2)
        ).astype(_F8_NP)
        x2n_c = np.ascontiguousarray(
            shard.reshape(CH, 128, C).transpose(1, 0, 2)
        ).astype(_F8_NP)
        in_maps.append({"rsb": rsb, "x2t": x2t_c, "x2n": x2n_c})
    return in_maps


def _merge(results, Wv, Wo, bo):
    Wv = np.asarray(Wv, np.float32)
    Wo = np.asarray(Wo, np.float32)
    bo = np.asarray(bo, np.float32)
    t_tot = np.zeros((H, C), np.float64)
    l_tot = np.zeros(H, np.float64)
    L_ROW = [0, 32, 64, 96]   # where group g's l column lives (see l-copies)
    for r in results:
        tt = r["tt"].astype(np.float64)                         # [128, 513]
        for g in range(NG):
            t_tot[:, g * NB : (g + 1) * NB] += tt[32 * g : 32 * g + H, :NB]
            l_tot += tt[L_ROW[g] : L_ROW[g] + H, NB] * PSCALE
    tn = t_tot / l_tot[:, None]                                 # [16, 2048]
    u = np.einsum("hc,hjc->hj", tn, Wv.astype(np.float64).reshape(H, J, C))
    out = u.reshape(HJ) @ Wo.T.astype(np.float64) + bo.astype(np.float64)
    return out.astype(np.float32).reshape(1, ODIM)


def kernel(x1, x2, Wq, Wk, Wv, Wo, bo):
    nc = _get_nc()
    in_maps = _prep_in_maps(x1, x2, Wq, Wk)
    res = run_bass_kernel_spmd(nc, in_maps, list(range(NCORES)))
    return _merge(res.results, Wv, Wo, bo)


def run_traced(x1, x2, Wq, Wk, Wv, Wo, bo, **trace_kwargs):
    """Like kernel() but returns (output, BassKernelResults) with NTFF trace."""
    nc = _get_nc()
    in_maps = _prep_in_maps(x1, x2, Wq, Wk)
    res = run_bass_kernel_spmd(
        nc, in_maps, list(range(NCORES)), trace=True, **trace_kwargs
    )
    return _merge(res.results, Wv, Wo, bo), res
